# revision 39
# baseline (speedup 1.0000x reference)
"""Trainium2 Bass kernel for the DrugEncoder GNN (2x GCNConv + GraphNorm + pool).

Self-contained: host-side index preprocessing + two SPMD Bass launches on 8
NeuronCores.

Math restructuring (vs the naive reference graph):
- GCN layer 1 aggregates in the 64-dim input space BEFORE the W1 matmul
  (aggregation and the linear map commute).
- GCN layer 2 + global mean pool collapse into `(P @ h2) @ W2 + b2` where
  P[g, r] = (1/n_g) * sum_{edges r->c, c in g} dis_c dis_r  (+ self loops)
  is index-only data built on the host. This removes the second edge
  aggregation entirely.

Layer-1 aggregation uses a *fixed-rounds* layout instead of on-device
gather/scatter: the host lays the (dis_r-prescaled, bf16) source features of
each target's incident edges out as a padded dense stream
msgs[target, feat, round], so the device aggregation is a sequential DMA plus
a free-axis reduce_sum per 128-target block, followed by a dis_c column scale.
No dma_gather (the per-index GpSimd descriptor cost dominated the previous
version) and no one-hot indicator matmuls.

Sharding: graphs are slotted (256-node slots, 32 graphs per core) so that all
per-graph and per-block structure is static and identical across the 8 cores
(SPMD); per-core variability lives in data streams only.

Launch 1: per-core GraphNorm1 (+ dis_r prescale) -> y shard, feature-packed
to all 128 partitions. Host reassembles the full node-major y table and
expands it into the per-core rounds streams (pure byte movement, like the
slotting/P/partial-sum host steps).
Launch 2: rounds reduce + dis_c scale + W1/ReLU, GraphNorm2, P-matmul pooling.
Host sums the 8 partials and adds b2.
"""
import os
import sys

sys.path.insert(0, "/opt/trn_rl_repo")

import numpy as np

import concourse.bacc as bacc
import concourse.bass as bass
import concourse.mybir as mybir
import concourse.tile as tile
from concourse.bass_utils import run_bass_kernel_spmd

F32 = mybir.dt.float32
BF16 = mybir.dt.bfloat16
AF = mybir.ActivationFunctionType
OP = mybir.AluOpType
AX = mybir.AxisListType

C = 8            # cores
G = 256          # graphs
SLOT = 256       # nodes per graph slot
GPC = G // C     # graphs per core (32)
NPC = GPC * SLOT  # slotted nodes per core (8192)
NBLK = NPC // 128  # node blocks per core (64)
HGN = NPC // 2   # packed free dim in launch 1 (4096)
D0, DH, DO = 64, 128, 64
EPS = 1e-5
BLK_PER_CHUNK = 8  # msgs stream chunk granularity

LAST_EXEC_NS = []  # filled per launch when BASS_TRACE is set


def _ensure_axon_hooks():
    """bass_utils imports antenv.axon_hooks when trace=True under axon; some
    images lack it. Provide it (with the ctypes NTFF hook when the axon .so
    supports profiling, else a None hook so tracing degrades gracefully)."""
    if "antenv.axon_hooks" not in sys.modules:
        import types
        try:
            import antenv
        except ImportError:
            return
        mod = types.ModuleType("antenv.axon_hooks")
        mod._hook = None
        mod.set_axon_ntff_profile_hook = lambda h: setattr(mod, "_hook", h)
        mod.get_axon_ntff_profile_hook = lambda: mod._hook
        sys.modules["antenv.axon_hooks"] = mod
        antenv.axon_hooks = mod
    mod = sys.modules["antenv.axon_hooks"]
    if mod.get_axon_ntff_profile_hook() is not None:
        return
    try:
        import contextlib
        import ctypes

        lib = ctypes.CDLL("/opt/axon/libaxon_pjrt.so")
        if not hasattr(lib, "axon_start_nrt_profile"):
            return
        lib.axon_start_nrt_profile.argtypes = [
            ctypes.POINTER(ctypes.c_int64), ctypes.c_size_t]
        lib.axon_start_nrt_profile.restype = ctypes.c_int64
        lib.axon_stop_nrt_profile.argtypes = [ctypes.c_char_p]
        lib.axon_stop_nrt_profile.restype = ctypes.c_int64

        @contextlib.contextmanager
        def _hook(output_dir, device_ids):
            import jax
            jax.devices()
            if device_ids:
                ids = (ctypes.c_int64 * len(device_ids))(*device_ids)
                rc = lib.axon_start_nrt_profile(ids, len(device_ids))
            else:
                rc = lib.axon_start_nrt_profile(None, 0)
            try:
                yield
            finally:
                if rc == 0:
                    lib.axon_stop_nrt_profile(output_dir.encode())

        mod.set_axon_ntff_profile_hook(_hook)
    except Exception:
        pass


# --------------------------------------------------------------------------
# Host-side preprocessing (index data only)
# --------------------------------------------------------------------------

def _graph_perm(counts):
    """Assign graphs to cores balancing node counts (greedy, largest first).
    Returns perm[g] = slot index (core*GPC + slot_in_core)."""
    order = np.argsort(-counts, kind="stable")
    loads = np.zeros(C, np.int64)
    fill = np.zeros(C, np.int64)
    perm = np.zeros(G, np.int64)
    for g in order:
        k = int(np.argmin(loads + np.where(fill >= GPC, 1 << 40, 0)))
        perm[g] = k * GPC + fill[k]
        fill[k] += 1
        loads[k] += counts[g]
    return perm


def _slot_nodes(batch):
    """slotted id = gperm[g]*SLOT + pos; gperm balances node counts per core."""
    counts = np.bincount(batch, minlength=G).astype(np.int64)
    assert counts.max() <= SLOT, f"graph size {counts.max()} > SLOT {SLOT}"
    gperm = _graph_perm(counts)
    starts = np.zeros(G + 1, np.int64)
    np.cumsum(counts, out=starts[1:])
    pos = np.arange(len(batch)) - starts[batch]
    slotted = gperm[batch] * SLOT + pos
    return slotted.astype(np.int64), counts, gperm


def _preprocess(edge_index, batch):
    N = batch.shape[0]
    row = np.asarray(edge_index[0], dtype=np.int64)
    col = np.asarray(edge_index[1], dtype=np.int64)
    batch = np.asarray(batch, dtype=np.int64)
    slotted, counts, gperm = _slot_nodes(batch)

    deg = np.bincount(col, minlength=N).astype(np.float64) + 1.0
    dis = (1.0 / np.sqrt(deg)).astype(np.float32)

    srow = slotted[row]
    scol = slotted[col]
    sdis = np.zeros(C * NPC, np.float32)
    sdis[slotted] = dis

    # K8-packed row layout: each target owns ceil(deg/8) rows of 8 round
    # slots; rows are target-major within each 128-target block and padded
    # to TB[b]*128 rows (TB maxed across cores for SPMD-static shapes).
    # A host-built indicator ind[row, t] = dis_c[t] (nonzero iff row belongs
    # to t) turns per-block row-sums into the dis_c-scaled aggT via one PE
    # matmul accumulation per row-tile.
    K8 = 8
    r_all = np.concatenate([srow, slotted])
    c_all = np.concatenate([scol, slotted])
    order = np.argsort(c_all, kind="stable")
    r_all, c_all = r_all[order], c_all[order]
    deg_all = np.bincount(c_all, minlength=C * NPC)
    starts = np.zeros(C * NPC + 1, np.int64)
    np.cumsum(deg_all, out=starts[1:])
    pos = np.arange(len(c_all)) - starts[c_all]

    nr_slot = (deg_all + K8 - 1) // K8             # rows per target
    nrb = nr_slot.reshape(C, NBLK, 128)
    rows_kb = nrb.sum(axis=2)                      # [C, NBLK]
    TB = np.maximum(1, -(-rows_kb.max(axis=0) // 128)).astype(np.int64)
    ROWS_MAX = int(TB.max()) * 128

    rowstart = np.zeros((C, NBLK, 128), np.int64)
    np.cumsum(nrb[:, :, :-1], axis=2, out=rowstart[:, :, 1:])

    k_e = c_all // NPC
    b_e = (c_all % NPC) // 128
    t_e = c_all % 128
    row_e = rowstart[k_e, b_e, t_e] + pos // K8
    sub_e = pos % K8
    src_rows = np.full((C, NBLK, ROWS_MAX, K8), C * NPC, np.int64)
    src_rows[k_e, b_e, row_e, sub_e] = r_all

    # indicator values: for each real target, its nr rows get sdis[target]
    ind = np.zeros((C, NBLK, ROWS_MAX, 128), np.float32)
    nr_flat = nr_slot  # slot-ordered == (k, b, t)-ordered
    tot_rows = int(nr_flat.sum())
    rep_slot = np.repeat(np.arange(C * NPC), nr_flat)
    rs_flat = rowstart.reshape(-1)
    row_off = np.arange(tot_rows) - np.repeat(
        np.concatenate([[0], np.cumsum(nr_flat)[:-1]]), nr_flat)
    rep_row = rs_flat[rep_slot] + row_off
    ind[rep_slot // NPC, (rep_slot % NPC) // 128, rep_row,
        rep_slot % 128] = sdis[rep_slot]

    moff = np.zeros(NBLK + 1, np.int64)
    np.cumsum(TB * D0 * K8, out=moff[1:])
    ioff = np.zeros(NBLK + 1, np.int64)
    np.cumsum(TB * 128, out=ioff[1:])

    return dict(
        slotted=slotted, counts=counts, gperm=gperm, dis=dis, sdis=sdis,
        K8=K8, TB=TB, moff=moff, ioff=ioff, src_rows=src_rows, ind=ind,
        batch=batch, row=row, col=col,
    )


def _build_P(pp):
    row, col, batch = pp["row"], pp["col"], pp["batch"]
    dis, counts, slotted = pp["dis"], pp["counts"], pp["slotted"]
    g_of_col = batch[col]
    w = dis[col].astype(np.float64) * dis[row].astype(np.float64)
    flat = g_of_col * (C * NPC) + slotted[row]
    P = np.bincount(flat, weights=w, minlength=G * C * NPC)
    flat2 = batch * (C * NPC) + slotted
    P += np.bincount(flat2, weights=dis.astype(np.float64) ** 2,
                     minlength=G * C * NPC)
    P = P.reshape(G, C * NPC)
    P /= np.maximum(counts[:, None], 1).astype(np.float64)
    return P.astype(np.float32)


# --------------------------------------------------------------------------
# Launch 1: GraphNorm1 + dis prescale -> y shard (feature-packed, 128 parts)
# --------------------------------------------------------------------------

def _build_launch1():
    nc = bacc.Bacc("TRN2", target_bir_lowering=False, debug=False)
    xT = nc.dram_tensor("xT", [128, HGN], BF16, kind="ExternalInput")
    dis2 = nc.dram_tensor("dis2", [128, HGN], BF16, kind="ExternalInput")
    invn = nc.dram_tensor("invn", [128, GPC // 2], F32, kind="ExternalInput")
    msv = nc.dram_tensor("msv", [128, 1], F32, kind="ExternalInput")
    wv = nc.dram_tensor("wv", [128, 1], F32, kind="ExternalInput")
    bv = nc.dram_tensor("bv", [128, 1], F32, kind="ExternalInput")
    epsv = nc.dram_tensor("epsv", [128, 1], F32, kind="ExternalInput")
    y_out = nc.dram_tensor("y_out", [128, HGN], BF16, kind="ExternalOutput")

    GH = GPC // 2  # 16 graph columns in packed layout
    with tile.TileContext(nc) as tc:
        with tc.tile_pool(name="sb", bufs=1) as sb:
            xT_sb = sb.tile([128, HGN], BF16)
            nc.sync.dma_start(out=xT_sb[:], in_=xT[:])
            dis_sb = sb.tile([128, HGN], BF16)
            nc.sync.dma_start(out=dis_sb[:], in_=dis2[:])
            invn_sb = sb.tile([128, GH], F32)
            nc.sync.dma_start(out=invn_sb[:], in_=invn[:])
            ms_sb = sb.tile([128, 1], F32)
            nc.sync.dma_start(out=ms_sb[:], in_=msv[:])
            w_sb = sb.tile([128, 1], F32)
            nc.sync.dma_start(out=w_sb[:], in_=wv[:])
            b_sb = sb.tile([128, 1], F32)
            nc.sync.dma_start(out=b_sb[:], in_=bv[:])
            eps_sb = sb.tile([128, 1], F32)
            nc.sync.dma_start(out=eps_sb[:], in_=epsv[:])

            sums = sb.tile([128, GH], F32)
            sumsq = sb.tile([128, GH], F32)
            nc.vector.reduce_sum(
                out=sums[:], in_=xT_sb[:].rearrange("p (g s) -> p g s", s=SLOT),
                axis=AX.X)
            for gs in range(GH):
                sq = sb.tile([128, SLOT], BF16, name=f"sq{gs}")
                nc.scalar.activation(
                    out=sq[:], in_=xT_sb[:, gs * SLOT:(gs + 1) * SLOT],
                    func=AF.Square, accum_out=sumsq[:, gs:gs + 1])

            mu = sb.tile([128, GH], F32)
            nc.vector.tensor_tensor(out=mu[:], in0=sums[:], in1=invn_sb[:],
                                    op=OP.mult)
            m2 = sb.tile([128, GH], F32)
            nc.vector.tensor_scalar(out=m2[:], in0=mu[:], scalar1=ms_sb[:, :1],
                                    scalar2=None, op0=OP.mult)
            ex2 = sb.tile([128, GH], F32)
            nc.vector.tensor_tensor(out=ex2[:], in0=sumsq[:], in1=invn_sb[:],
                                    op=OP.mult)
            var = sb.tile([128, GH], F32)
            nc.vector.tensor_tensor(out=var[:], in0=m2[:], in1=mu[:], op=OP.mult)
            nc.vector.tensor_scalar(out=var[:], in0=var[:], scalar1=-2.0,
                                    scalar2=None, op0=OP.mult)
            nc.vector.tensor_tensor(out=var[:], in0=var[:], in1=ex2[:], op=OP.add)
            m2sq = sb.tile([128, GH], F32)
            nc.vector.tensor_tensor(out=m2sq[:], in0=m2[:], in1=m2[:], op=OP.mult)
            nc.vector.tensor_tensor(out=var[:], in0=var[:], in1=m2sq[:], op=OP.add)
            std = sb.tile([128, GH], F32)
            nc.scalar.activation(out=std[:], in_=var[:], func=AF.Sqrt,
                                 bias=eps_sb[:, :1])
            inv = sb.tile([128, GH], F32)
            nc.vector.reciprocal(out=inv[:], in_=std[:])
            Av = sb.tile([128, GH], F32)
            nc.vector.tensor_scalar(out=Av[:], in0=inv[:], scalar1=w_sb[:, :1],
                                    scalar2=None, op0=OP.mult)
            Bv = sb.tile([128, GH], F32)
            nc.vector.tensor_tensor(out=Bv[:], in0=Av[:], in1=m2[:], op=OP.mult)
            nc.vector.tensor_scalar(out=Bv[:], in0=Bv[:], scalar1=-1.0,
                                    scalar2=b_sb[:, :1], op0=OP.mult, op1=OP.add)

            h0 = sb.tile([128, HGN], BF16)
            for gs in range(GH):
                nc.vector.tensor_scalar(
                    out=h0[:, gs * SLOT:(gs + 1) * SLOT],
                    in0=xT_sb[:, gs * SLOT:(gs + 1) * SLOT],
                    scalar1=Av[:, gs:gs + 1], scalar2=Bv[:, gs:gs + 1],
                    op0=OP.mult, op1=OP.add)
            y_sb = sb.tile([128, HGN], BF16)
            nc.vector.tensor_tensor(out=y_sb[:], in0=h0[:], in1=dis_sb[:],
                                    op=OP.mult)
            nc.sync.dma_start(out=y_out[:], in_=y_sb[:])
    nc.compile()
    return nc


# --------------------------------------------------------------------------
# Launch 2: rounds reduce + dis_c + W1/ReLU + GraphNorm2 + P-matmul pooling
# --------------------------------------------------------------------------

def _build_launch2(pp):
    K8, TB, moff, ioff = pp["K8"], pp["TB"], pp["moff"], pp["ioff"]
    NB4 = BLK_PER_CHUNK
    NCHUNK = NBLK // NB4
    chunk_m = [(int(moff[c * NB4]), int(moff[(c + 1) * NB4]))
               for c in range(NCHUNK)]
    chunk_i = [(int(ioff[c * NB4]), int(ioff[(c + 1) * NB4]))
               for c in range(NCHUNK)]
    CHM = max(b - a for a, b in chunk_m)
    CHI = max(b - a for a, b in chunk_i)
    FREE_M, FREE_I = int(moff[-1]), int(ioff[-1])

    nc = bacc.Bacc("TRN2", target_bir_lowering=False, debug=False)
    msgs = nc.dram_tensor("msgs", [128, FREE_M], BF16, kind="ExternalInput")
    inds = nc.dram_tensor("inds", [128, FREE_I], BF16, kind="ExternalInput")
    ident = nc.dram_tensor("ident", [128, 128], BF16, kind="ExternalInput")
    PT = nc.dram_tensor("PT", [NPC, G], BF16, kind="ExternalInput")
    W1 = nc.dram_tensor("W1", [D0, DH], BF16, kind="ExternalInput")
    b1 = nc.dram_tensor("b1", [DH, 1], F32, kind="ExternalInput")
    W2 = nc.dram_tensor("W2", [DH, DO], F32, kind="ExternalInput")
    gn2w = nc.dram_tensor("gn2w", [DH, 1], F32, kind="ExternalInput")
    gn2b = nc.dram_tensor("gn2b", [DH, 1], F32, kind="ExternalInput")
    gn2ms = nc.dram_tensor("gn2ms", [DH, 1], F32, kind="ExternalInput")
    invn2 = nc.dram_tensor("invn2", [DH, GPC], F32, kind="ExternalInput")
    npad = nc.dram_tensor("npad", [DH, GPC], F32, kind="ExternalInput")
    epsv = nc.dram_tensor("epsv", [DH, 1], F32, kind="ExternalInput")
    part = nc.dram_tensor("part", [G, DO], F32, kind="ExternalOutput")

    with tile.TileContext(nc) as tc:
        with tc.tile_pool(name="cst", bufs=1) as cst:
            id_sb = cst.tile([128, 128], BF16)
            W1_sb = cst.tile([D0, DH], BF16)
            b1_sb = cst.tile([DH, 1], F32)
            W2_sb = cst.tile([DH, DO], F32)
            gn2w_sb = cst.tile([DH, 1], F32)
            gn2b_sb = cst.tile([DH, 1], F32)
            gn2ms_sb = cst.tile([DH, 1], F32)
            invn2_sb = cst.tile([DH, GPC], F32)
            npad_sb = cst.tile([DH, GPC], F32)
            eps_sb = cst.tile([DH, 1], F32)
            # PT prefetch tile (DMA issued mid-stream)
            PT_sb = cst.tile([128, NBLK, G], BF16)

            def emit_const_dmas():
                nc.sync.dma_start(out=id_sb[:], in_=ident[:])
                nc.sync.dma_start(out=W1_sb[:], in_=W1[:])
                nc.sync.dma_start(out=b1_sb[:], in_=b1[:])
                nc.sync.dma_start(out=W2_sb[:], in_=W2[:])
                nc.sync.dma_start(out=gn2w_sb[:], in_=gn2w[:])
                nc.sync.dma_start(out=gn2b_sb[:], in_=gn2b[:])
                nc.sync.dma_start(out=gn2ms_sb[:], in_=gn2ms[:])
                nc.sync.dma_start(out=invn2_sb[:], in_=invn2[:])
                nc.sync.dma_start(out=npad_sb[:], in_=npad[:])
                nc.sync.dma_start(out=eps_sb[:], in_=epsv[:])

            relu_b1 = cst.tile([DH, 1], F32)
            relu_b1sq = cst.tile([DH, 1], F32)

            h1T = cst.tile([DH, NPC], BF16)
            h2bf = cst.tile([DH, NPC], BF16)
            sums = cst.tile([DH, GPC], F32)
            sumsq = cst.tile([DH, GPC], F32)

            # ---------------- K8-row reduce + indicator matmul + W1/ReLU ----
            # Per chunk of 4 blocks (= 2 graph slots): 3-level bf16
            # tensor_tensor tree (2x DVE mode) sums each row's 8 round slots,
            # then per block a PSUM-accumulated matmul against the host-built
            # indicator (values dis_c) turns row-sums into the feature-major,
            # dis_c-scaled aggT; then W1 matmul + fused bias/ReLU on Act.
            # GraphNorm2 statistics accumulate incrementally per chunk, and
            # the GraphNorm2 apply + pooling run in two halves so the first
            # half overlaps the second half of the msgs stream.
            HALF_G = GPC // 2
            with tc.tile_pool(name="msg", bufs=3) as msgp, \
                 tc.tile_pool(name="ind", bufs=3) as indp, \
                 tc.tile_pool(name="tree", bufs=2) as treep, \
                 tc.tile_pool(name="agg", bufs=4) as aggp, \
                 tc.tile_pool(name="sqs", bufs=2) as sqsp, \
                 tc.tile_pool(name="gn", bufs=2) as gn, \
                 tc.tile_pool(name="pe", bufs=4) as pe, \
                 tc.tile_pool(name="trps", bufs=2, space="PSUM") as trps, \
                 tc.tile_pool(name="h1ps", bufs=2, space="PSUM") as h1psp, \
                 tc.tile_pool(name="peps", bufs=2, space="PSUM") as peps, \
                 tc.tile_pool(name="poolps", bufs=1, space="PSUM") as poolps, \
                 tc.tile_pool(name="outps", bufs=1, space="PSUM") as outps:
                poolT = poolps.tile([DH, G], F32, tag="poolT")
                for ci in range(NCHUNK):
                    b0 = ci * NB4
                    m0, m1 = chunk_m[ci]
                    i0, i1 = chunk_i[ci]
                    Q = (m1 - m0) // K8
                    ch = msgp.tile([128, CHM], BF16, tag="ch")
                    nc.sync.dma_start(out=ch[:, :m1 - m0], in_=msgs[:, m0:m1])
                    ich = indp.tile([128, CHI], BF16, tag="ich")
                    nc.sync.dma_start(out=ich[:, :i1 - i0],
                                      in_=inds[:, i0:i1])
                    if ci == 0:
                        # small consts ride the SP queue behind chunk 0
                        emit_const_dmas()
                        nc.scalar.activation(out=relu_b1[:], in_=b1_sb[:],
                                             func=AF.Relu)
                        nc.vector.tensor_tensor(out=relu_b1sq[:],
                                                in0=relu_b1[:],
                                                in1=relu_b1[:], op=OP.mult)
                    if ci == 2:
                        # PT prefetch, after the first pipeline is primed but
                        # well before the first pool-group needs it
                        nc.sync.dma_start(
                            out=PT_sb[:],
                            in_=PT.rearrange("(t p) g -> p t g", p=128))
                    ch3 = ch[:, :m1 - m0].rearrange("p (q r) -> p q r", r=K8)
                    t4 = treep.tile([128, CHM // K8, 4], BF16, tag="t4")
                    nc.vector.tensor_tensor(out=t4[:, :Q, :],
                                            in0=ch3[:, :, 0:4],
                                            in1=ch3[:, :, 4:8], op=OP.add)
                    t2 = treep.tile([128, CHM // K8, 2], BF16, tag="t2")
                    nc.vector.tensor_tensor(out=t2[:, :Q, :],
                                            in0=t4[:, :Q, 0:2],
                                            in1=t4[:, :Q, 2:4], op=OP.add)
                    rsum = aggp.tile([128, CHM // K8], BF16, tag="rsum")
                    nc.vector.tensor_tensor(
                        out=rsum[:, :Q].rearrange("p (q r) -> p q r", r=1),
                        in0=t2[:, :Q, 0:1], in1=t2[:, :Q, 1:2], op=OP.add)
                    jb = 0
                    for b in range(b0, b0 + NB4):
                        TBb = int(TB[b])
                        aggTps = trps.tile([D0, 128], F32, tag="trp")
                        for j in range(TBb):
                            q0 = (jb + j) * D0
                            nc.tensor.matmul(
                                out=aggTps[:], lhsT=rsum[:, q0:q0 + D0],
                                rhs=ich[:, (jb + j) * 128:(jb + j + 1) * 128],
                                start=(j == 0), stop=(j == TBb - 1))
                        jb += TBb
                        aggT = aggp.tile([D0, 128], BF16, tag="aggT")
                        nc.scalar.activation(out=aggT[:], in_=aggTps[:],
                                             func=AF.Copy)
                        h1ps = h1psp.tile([DH, 128], F32, tag="h1ps")
                        nc.tensor.matmul(out=h1ps[:], lhsT=W1_sb[:],
                                         rhs=aggT[:], start=True, stop=True)
                        nc.scalar.activation(
                            out=h1T[:, b * 128:(b + 1) * 128], in_=h1ps[:],
                            func=AF.Relu, bias=b1_sb[:, :1])
                    # incremental GraphNorm2 stats for this chunk's graphs
                    GPCH = NB4 // 2
                    g2 = GPCH * ci
                    nc.vector.reduce_sum(
                        out=sums[:, g2:g2 + GPCH],
                        in_=h1T[:, ci * NB4 * 128:
                                (ci + 1) * NB4 * 128].rearrange(
                            "p (g s) -> p g s", s=SLOT),
                        axis=AX.X)
                    for gi in range(GPCH):
                        gs = g2 + gi
                        sq = sqsp.tile([DH, SLOT], BF16, tag="sq")
                        nc.scalar.activation(
                            out=sq[:], in_=h1T[:, gs * SLOT:(gs + 1) * SLOT],
                            func=AF.Square,
                            accum_out=sumsq[:, gs:gs + 1])

                    split_plan = {NCHUNK // 2 - 1: slice(0, GPC // 2),
                                  3 * NCHUNK // 4 - 1:
                                      slice(GPC // 2, 3 * GPC // 4),
                                  NCHUNK - 2:
                                      slice(3 * GPC // 4, 7 * GPC // 8),
                                  NCHUNK - 1: slice(7 * GPC // 8, GPC)}
                    if ci not in split_plan:
                        continue
                    # -------- GraphNorm2 stats math + apply + pool --------
                    # for the group of graphs whose stats just completed
                    hs = split_plan[ci]
                    HG = hs.stop - hs.start
                    corr = gn.tile([DH, HG], F32, tag="corr")
                    nc.vector.tensor_scalar(out=corr[:], in0=npad_sb[:, hs],
                                            scalar1=relu_b1[:, :1],
                                            scalar2=None, op0=OP.mult)
                    nc.vector.tensor_tensor(out=sums[:, hs], in0=sums[:, hs],
                                            in1=corr[:], op=OP.subtract)
                    nc.vector.tensor_scalar(out=corr[:], in0=npad_sb[:, hs],
                                            scalar1=relu_b1sq[:, :1],
                                            scalar2=None, op0=OP.mult)
                    nc.vector.tensor_tensor(out=sumsq[:, hs],
                                            in0=sumsq[:, hs],
                                            in1=corr[:], op=OP.subtract)
                    mu = gn.tile([DH, HG], F32, tag="mu")
                    nc.vector.tensor_tensor(out=mu[:], in0=sums[:, hs],
                                            in1=invn2_sb[:, hs], op=OP.mult)
                    m2 = gn.tile([DH, HG], F32, tag="m2")
                    nc.vector.tensor_scalar(out=m2[:], in0=mu[:],
                                            scalar1=gn2ms_sb[:, :1],
                                            scalar2=None, op0=OP.mult)
                    ex2 = gn.tile([DH, HG], F32, tag="ex2")
                    nc.vector.tensor_tensor(out=ex2[:], in0=sumsq[:, hs],
                                            in1=invn2_sb[:, hs], op=OP.mult)
                    var = gn.tile([DH, HG], F32, tag="var")
                    nc.vector.tensor_tensor(out=var[:], in0=m2[:], in1=mu[:],
                                            op=OP.mult)
                    nc.vector.tensor_scalar(out=var[:], in0=var[:],
                                            scalar1=-2.0,
                                            scalar2=None, op0=OP.mult)
                    nc.vector.tensor_tensor(out=var[:], in0=var[:],
                                            in1=ex2[:], op=OP.add)
                    m2sq = gn.tile([DH, HG], F32, tag="m2sq")
                    nc.vector.tensor_tensor(out=m2sq[:], in0=m2[:], in1=m2[:],
                                            op=OP.mult)
                    nc.vector.tensor_tensor(out=var[:], in0=var[:],
                                            in1=m2sq[:], op=OP.add)
                    std = gn.tile([DH, HG], F32, tag="std")
                    nc.scalar.activation(out=std[:], in_=var[:], func=AF.Sqrt,
                                         bias=eps_sb[:, :1])
                    inv = gn.tile([DH, HG], F32, tag="inv")
                    nc.vector.reciprocal(out=inv[:], in_=std[:])
                    Av = gn.tile([DH, HG], F32, tag="Av")
                    nc.vector.tensor_scalar(out=Av[:], in0=inv[:],
                                            scalar1=gn2w_sb[:, :1],
                                            scalar2=None, op0=OP.mult)
                    Bv = gn.tile([DH, HG], F32, tag="Bv")
                    nc.vector.tensor_tensor(out=Bv[:], in0=Av[:], in1=m2[:],
                                            op=OP.mult)
                    nc.vector.tensor_scalar(out=Bv[:], in0=Bv[:], scalar1=-1.0,
                                            scalar2=gn2b_sb[:, :1],
                                            op0=OP.mult, op1=OP.add)
                    for gj in range(HG):
                        gs = hs.start + gj
                        nc.vector.tensor_scalar(
                            out=h2bf[:, gs * SLOT:(gs + 1) * SLOT],
                            in0=h1T[:, gs * SLOT:(gs + 1) * SLOT],
                            scalar1=Av[:, gj:gj + 1], scalar2=Bv[:, gj:gj + 1],
                            op0=OP.mult, op1=OP.add)
                        for cki in (2 * gs, 2 * gs + 1):
                            trp2 = peps.tile([128, 128], BF16, tag="trp2")
                            nc.tensor.transpose(
                                out=trp2[:],
                                in_=h2bf[:, cki * 128:(cki + 1) * 128],
                                identity=id_sb[:])
                            h2nm = pe.tile([128, 128], BF16, tag="h2nm")
                            ceng = nc.vector if cki % 2 else nc.scalar
                            if ceng is nc.vector:
                                nc.vector.tensor_copy(out=h2nm[:], in_=trp2[:])
                            else:
                                nc.scalar.activation(out=h2nm[:], in_=trp2[:],
                                                     func=AF.Copy)
                            nc.tensor.matmul(out=poolT[:], lhsT=h2nm[:],
                                             rhs=PT_sb[:, cki, :],
                                             start=(cki == 0),
                                             stop=(cki == NBLK - 1))
                poolT_sb = pe.tile([DH, G], F32, tag="poolTsb")
                nc.vector.tensor_copy(out=poolT_sb[:], in_=poolT[:])
                for hh in range(2):
                    ops_ = outps.tile([128, DO], F32, tag="ops")
                    nc.tensor.matmul(
                        out=ops_[:],
                        lhsT=poolT_sb[:, hh * 128:(hh + 1) * 128],
                        rhs=W2_sb[:], start=True, stop=True)
                    out_sb = pe.tile([128, DO], F32, tag="outsb")
                    nc.vector.tensor_copy(out=out_sb[:], in_=ops_[:])
                    nc.sync.dma_start(out=part[hh * 128:(hh + 1) * 128, :],
                                      in_=out_sb[:])
    nc.compile()
    return nc


# --------------------------------------------------------------------------
# Entry point
# --------------------------------------------------------------------------

def kernel(**inputs):
    global LAST_EXEC_NS
    LAST_EXEC_NS = []
    import ml_dtypes

    x = np.asarray(inputs["x"], np.float32)
    edge_index = np.asarray(inputs["edge_index"])
    batch = np.asarray(inputs["batch"])
    gn1_w = np.asarray(inputs["gn1_w"], np.float32)
    gn1_b = np.asarray(inputs["gn1_b"], np.float32)
    gn1_ms = np.asarray(inputs["gn1_ms"], np.float32)
    W1 = np.asarray(inputs["W1"], np.float32)
    b1 = np.asarray(inputs["b1"], np.float32)
    gn2_w = np.asarray(inputs["gn2_w"], np.float32)
    gn2_b = np.asarray(inputs["gn2_b"], np.float32)
    gn2_ms = np.asarray(inputs["gn2_ms"], np.float32)
    W2 = np.asarray(inputs["W2"], np.float32)
    b2 = np.asarray(inputs["b2"], np.float32)

    trace = bool(os.environ.get("BASS_TRACE"))
    if trace:
        _ensure_axon_hooks()

    pp = _preprocess(edge_index, batch)
    P = _build_P(pp)
    counts, slotted, sdis = pp["counts"], pp["slotted"], pp["sdis"]
    invperm = np.argsort(pp["gperm"])  # slot -> original graph
    slot_counts = counts[invperm]      # counts ordered by slot

    # slotted x
    xs = np.zeros((C * NPC, D0), np.float32)
    xs[slotted] = x
    ident = np.eye(128, dtype=ml_dtypes.bfloat16)

    def pack2(a_k):
        # [NPC, D0] node-major -> [128, HGN] feature-packed two halves
        return np.ascontiguousarray(
            a_k.reshape(2, HGN, D0).transpose(0, 2, 1).reshape(128, HGN))

    # ---- launch 1 ----
    nc1 = _build_launch1()
    in_maps1 = []
    GH = GPC // 2
    for k in range(C):
        xT_k = pack2(xs[k * NPC:(k + 1) * NPC]).astype(ml_dtypes.bfloat16)
        dis_k = pack2(np.broadcast_to(
            sdis[k * NPC:(k + 1) * NPC][:, None],
            (NPC, D0))).astype(ml_dtypes.bfloat16)
        n_k = slot_counts[k * GPC:(k + 1) * GPC].astype(np.float64)
        inv_n = (1.0 / np.maximum(n_k, 1.0)).astype(np.float32)  # [32]
        invn_k = np.empty((128, GH), np.float32)
        invn_k[:D0] = inv_n[:GH][None, :]
        invn_k[D0:] = inv_n[GH:][None, :]
        in_maps1.append({
            "xT": xT_k, "dis2": dis_k, "invn": invn_k,
            "msv": np.tile(gn1_ms, 2)[:, None].copy(),
            "wv": np.tile(gn1_w, 2)[:, None].copy(),
            "bv": np.tile(gn1_b, 2)[:, None].copy(),
            "epsv": np.full((128, 1), EPS, np.float32),
        })
    res1 = run_bass_kernel_spmd(nc1, in_maps1, core_ids=list(range(C)),
                                trace=trace)
    if res1.exec_time_ns is not None:
        LAST_EXEC_NS.append(res1.exec_time_ns)

    # unpack y into the global node-major table (+ zero row for pads)
    y_pad = np.zeros((C * NPC + 1, D0), ml_dtypes.bfloat16)
    for k in range(C):
        y2 = np.asarray(res1.results[k]["y_out"])  # [128, HGN] bf16
        y_pad[k * NPC:(k + 1) * NPC] = (
            y2.reshape(2, D0, HGN).transpose(0, 2, 1).reshape(NPC, D0))

    # K8-packed row streams + indicator streams
    K8, TB, moff, ioff = pp["K8"], pp["TB"], pp["moff"], pp["ioff"]
    src_rows, ind = pp["src_rows"], pp["ind"]
    FREE_M, FREE_I = int(moff[-1]), int(ioff[-1])
    msgs_all, inds_all = [], []
    for k in range(C):
        m_k = np.empty((128, FREE_M), ml_dtypes.bfloat16)
        i_k = np.empty((128, FREE_I), ml_dtypes.bfloat16)
        for b in range(NBLK):
            TBb = int(TB[b])
            rows = TBb * 128
            mb = y_pad[src_rows[k, b, :rows].reshape(-1)]  # [rows*K8, D0]
            mb = mb.reshape(TBb, 128, K8, D0).transpose(1, 0, 3, 2)
            m_k[:, moff[b]:moff[b] + TBb * D0 * K8] = (
                mb.reshape(128, TBb * D0 * K8))
            ib = ind[k, b, :rows].reshape(TBb, 128, 128).transpose(1, 0, 2)
            i_k[:, ioff[b]:ioff[b] + TBb * 128] = (
                ib.reshape(128, TBb * 128).astype(ml_dtypes.bfloat16))
        msgs_all.append(m_k)
        inds_all.append(i_k)

    # ---- launch 2 ----
    nc2 = _build_launch2(pp)
    in_maps2 = []
    for k in range(C):
        n_k = slot_counts[k * GPC:(k + 1) * GPC].astype(np.float64)
        invn2_k = np.broadcast_to(
            (1.0 / np.maximum(n_k, 1.0)).astype(np.float32)[None, :],
            (DH, GPC)).copy()
        npad_k = np.broadcast_to(
            (SLOT - n_k).astype(np.float32)[None, :], (DH, GPC)).copy()
        PT_k = np.ascontiguousarray(
            P[:, k * NPC:(k + 1) * NPC].T.astype(ml_dtypes.bfloat16))
        in_maps2.append({
            "msgs": msgs_all[k], "inds": inds_all[k],
            "ident": ident, "PT": PT_k,
            "W1": W1.astype(ml_dtypes.bfloat16),
            "b1": b1[:, None].copy(), "W2": W2,
            "gn2w": gn2_w[:, None].copy(), "gn2b": gn2_b[:, None].copy(),
            "gn2ms": gn2_ms[:, None].copy(),
            "invn2": invn2_k, "npad": npad_k,
            "epsv": np.full((DH, 1), EPS, np.float32),
        })
    res2 = run_bass_kernel_spmd(nc2, in_maps2, core_ids=list(range(C)),
                                trace=trace)
    if res2.exec_time_ns is not None:
        LAST_EXEC_NS.append(res2.exec_time_ns)
    out = np.sum([res2.results[k]["part"] for k in range(C)], axis=0)
    out = out + b2[None, :]
    return out.astype(np.float32)
